# revision 32
# baseline (speedup 1.0000x reference)
"""GCN (MLP pre-encoder + 2 GCNConv layers) on 8 Trainium2 NeuronCores.

Strategy (graph/data parallel, per sharding hint):
- Nodes sharded by rows across 8 cores (12500 -> padded 12544 each).
- Edges partitioned by destination core, sorted by (dest tile-group,
  source segment, dest tile); self-loops appended as ordinary edges.
- Symmetric normalization folded into the gathered feature tables:
  g' = dis * g, so each message is an unweighted row gather and the
  per-dest scale dis[d] is applied once per output row.
- Aggregation per 128-dest tile: dma_gather rows of the AllGathered
  table, scatter-add within the tile via a one-hot matmul on the PE
  (onehot[m, d] = (dest_local[m] == d)), accumulated in PSUM.
- v2: phase A runs in bf16; gathers merged across TF dest tiles per
  instruction; PSUM->SBUF copies and scaling on the Activation engine.
- v3: each AllGather is split into two half-node-range collectives
  (tables A/B) so communication overlaps compute: AG(A-half) runs
  while phase A computes the B half; aggregation runs in two passes
  (sources in half A, then half B), with the pass-1 partial parked in
  SBUF and merged back into PSUM via an identity matmul, so pass 1
  overlaps the second collective.
- v4 (5.8ms -> ~2.3ms): the SWDGE descriptor ring was the choke: at
  the default 16KB scratch (1024 descs) a ~1131-desc gather cannot
  even fit, so each gather's ring-space wait serialized desc-gen
  behind the FULL drain of every prior same-queue gather, and the DMA
  stream never overlapped compute.  dynamic_dma_scratch_size=49152
  (3072 descs) keeps 2-3 gathers in flight per queue.  One-hot
  builds are merged (one broadcast tensor_tensor is_equal per (tile,
  seg) builds all its 128-edge chunks; dloc stored bf16) cutting DVE
  instructions ~9x.  The pass-0 partial is parked in DRAM via the
  idle HWDGE queues instead of a 50KB SBUF slab, freeing SBUF for
  msgs double-buffering (6 bufs).  Phase E computes at CE=64 columns
  (N_CLS=40) while gather rows stay at the 256B minimum.  Gathers are
  per-descriptor latency-bound (~6.5ns/desc/queue, 4 queues), so the
  ~1.35ms two-phase gather stream is the roofline; compute hides
  under it almost completely.
"""
import os
import sys
sys.path.insert(0, "/opt/trn_rl_repo")
import numpy as np
import ml_dtypes
import concourse.bass as bass
import concourse.bacc as bacc
import concourse.mybir as mybir
from concourse import tile
from concourse.bass_utils import run_bass_kernel_spmd
from contextlib import ExitStack

N_NODES = 100000
N_FEAT = 512
H_MLP = 256
H_GCN = 256
N_CLS = 40
NCORES = 8
RPC = 12500         # real rows per core
RPAD = 12544        # padded rows per core (98 * 128)
NTILE = 98          # dest tiles per core
NSEG = 4
TF = int(os.environ.get("KTF", "1"))  # dest tiles merged per gather instruction
NTG = (NTILE + TF - 1) // TF
# half split: rows [0, RA) -> table A, rows [RA, RPC) -> table B
RA = 6144           # 48 dest tiles (phase-A RT=512 boundary: 12 iters)
RB = RPAD - RA      # 6400 rows, 50 tiles
TILES_A = RA // 128
TA_ROWS = NCORES * RA    # 49152 rows in table A
TB_ROWS = NCORES * RB    # 51200 rows in table B
SEGR_A = TA_ROWS // 2    # 24576 (< 2**15)
SEGR_B = TB_ROWS // 2    # 25600 (< 2**15)
MM_DT = mybir.dt.bfloat16
MM_NP = ml_dtypes.bfloat16
K_PHASES = int(os.environ.get("K_PHASES", "5"))  # debug: stop after N phases
NQ = int(os.environ.get("KQUEUES", "4"))        # SWDGE queues for gathers
KSP = os.environ.get("KSP", "0") == "1"         # single_packet mode
MBUFS = int(os.environ.get("KMBUFS", "6"))      # msgs tile-pool buffers
# SWDGE descriptor-ring scratch: 49152B = 3072 descs, so 2-3 gathers
# (~1131 descs each) can be in flight per queue instead of 0-1 at the
# 16384B default (ring-space wait serialized desc-gen behind the full
# drain of every previous same-queue gather).
KSCRATCH = int(os.environ.get("KSCRATCH", "49152"))
MERGEOH = os.environ.get("KMERGEOH", "1") == "1"  # one-hot build per (t,s)
SKIP_GATHER = os.environ.get("KSKIP_GATHER", "0") == "1"   # timing ablation
SKIP_ONEHOT = os.environ.get("KSKIP_ONEHOT", "0") == "1"   # timing ablation
C_PAD = 128         # N_CLS padded so gather rows are 256B
CE = 64             # phase-E compute width (N_CLS=40 padded to 64)


def _preprocess(edge_index):
    """Host-side graph preprocessing -> per-core edge structures."""
    row = np.asarray(edge_index[0], dtype=np.int64)
    col = np.asarray(edge_index[1], dtype=np.int64)
    deg = np.bincount(col, minlength=N_NODES).astype(np.float32) + 1.0
    dis = (1.0 / np.sqrt(deg)).astype(np.float32)

    allr = np.arange(N_NODES, dtype=np.int64)
    dest = np.concatenate([row, allr])
    src = np.concatenate([col, allr])

    core = dest // RPC
    dlocal = dest % RPC
    tile_id = dlocal // 128
    dl = (dlocal % 128).astype(np.float32)

    # source -> (half table, row, segment, int16 offset)
    sc = src // RPC
    sr = src % RPC
    h = (sr >= RA).astype(np.int64)
    trow = np.where(h == 0, sc * RA + sr, sc * RB + (sr - RA))
    segr = np.where(h == 0, SEGR_A, SEGR_B)
    seg = 2 * h + (trow >= segr)
    sloc = (trow - (trow >= segr) * segr).astype(np.int16)

    # per-(tile, seg) group sizes, padded to 16 (gather descriptors are
    # per-row and latency-bound, so padding is pure waste; the one-hot
    # masks the <=15 pad edges and the stale tail of the last 128-chunk),
    # shared across cores (SPMD)
    ngrp = NTILE * NSEG
    key_ts = (core * ngrp + tile_id * NSEG + seg).astype(np.int64)
    cnt = np.bincount(key_ts, minlength=NCORES * ngrp).reshape(NCORES, NTILE, NSEG)
    G = cnt.max(axis=0)
    G = ((G + 15) // 16) * 16             # [NTILE, NSEG]

    # storage order: (tg, s, t-within-tg); group offsets in that order
    order_keys = []
    for tg in range(NTG):
        tiles = range(tg * TF, min(tg * TF + TF, NTILE))
        for s in range(NSEG):
            for t in tiles:
                order_keys.append((t, s))
    sizes = np.array([G[t, s] for (t, s) in order_keys], np.int64)
    offs = np.concatenate([[0], np.cumsum(sizes)])[:-1]
    off_ts = np.zeros((NTILE, NSEG), np.int64)
    for (t, s), o in zip(order_keys, offs):
        off_ts[t, s] = o
    total = int(sizes.sum())

    # rank of each edge within its (core, t, s) group
    order = np.argsort(key_ts, kind="stable")
    sk = key_ts[order]
    starts = np.searchsorted(sk, np.arange(NCORES * ngrp))
    rank = np.arange(len(sk)) - starts[sk]
    ckey = sk // ngrp
    tkey = (sk % ngrp) // NSEG
    skey = sk % NSEG
    pos = off_ts[tkey, skey] + rank

    idx_arr = np.full((NCORES, total), -1, np.int16)   # pad -> skipped
    idx_arr[ckey, pos] = sloc[order]
    gidx = [np.tile(idx_arr[c].reshape(-1, 16).T, (8, 1)).copy() for c in range(NCORES)]

    # dest-in-tile table in a separate 128-chunk-padded layout (group offsets
    # are only 16-aligned; one-hot columns must be 128-edge chunks)
    CH = (G + 127) // 128                  # chunks per (t, s)
    chunk_off = np.zeros((NTILE, NSEG), np.int64)
    o = 0
    for (t, s) in order_keys:
        chunk_off[t, s] = o
        o += int(CH[t, s])
    total_ch = o
    dl_arr = np.full((NCORES, total_ch * 128), 255.0, np.float32)
    posc = chunk_off[tkey, skey] * 128 + rank
    dl_arr[ckey, posc] = dl[order]
    dloc = [np.ascontiguousarray(dl_arr[c].reshape(-1, 128).T) for c in range(NCORES)]

    dis_pad = np.ones(NCORES * RPAD, np.float32)
    for c in range(NCORES):
        dis_pad[c * RPAD:c * RPAD + RPC] = dis[c * RPC:(c + 1) * RPC]
    diso = [np.ascontiguousarray(
        dis_pad[c * RPAD:(c + 1) * RPAD].reshape(NTILE, 128).T) for c in range(NCORES)]

    return G, off_ts, chunk_off, total, gidx, dloc, diso, cnt


def _build_program(G, off_ts, chunk_off, total, b1_nonzero=False, reps=1,
                   k_phases=None):
    NGC = NTILE * NSEG  # gather-count columns per aggregation phase
    """Build the SPMD Bass program (identical across cores)."""
    K_PHASES = globals()["K_PHASES"] if k_phases is None else k_phases
    total16 = total // 16
    CH = (G + 127) // 128                  # chunks per (t, s)
    nch_tot = int(CH.sum())
    CHMAX = int(CH.max())
    N_tg = np.zeros((NTG, NSEG), np.int64)
    for tg in range(NTG):
        tiles = range(tg * TF, min(tg * TF + TF, NTILE))
        for s in range(NSEG):
            N_tg[tg, s] = sum(int(G[t, s]) for t in tiles)
    cap = max(1, int(((N_tg + 127) // 128).max()))   # msgs capacity (chunks)

    f32 = mybir.dt.float32
    nc = bacc.Bacc("TRN2", target_bir_lowering=False, debug=False,
                   num_devices=NCORES, num_swdge_queues=NQ,
                   dynamic_dma_scratch_size=KSCRATCH)

    # inputs
    t_xT = nc.dram_tensor("xT", [N_FEAT, RPAD], MM_DT, kind="ExternalInput")
    t_wmlp = nc.dram_tensor("wmlp", [N_FEAT, H_MLP], MM_DT, kind="ExternalInput")
    t_w1 = nc.dram_tensor("w1", [H_MLP, H_GCN], MM_DT, kind="ExternalInput")
    t_w2 = nc.dram_tensor("w2", [H_GCN, CE], MM_DT, kind="ExternalInput")
    t_bmlp = nc.dram_tensor("bmlp", [128, 2], f32, kind="ExternalInput")
    t_b1 = nc.dram_tensor("b1bc", [128, H_GCN], f32, kind="ExternalInput")
    t_b2 = nc.dram_tensor("b2bc", [128, CE], f32, kind="ExternalInput")
    t_iota = nc.dram_tensor("iota", [128, CHMAX * 128], MM_DT,
                            kind="ExternalInput")
    t_ident = nc.dram_tensor("ident", [128, 128], MM_DT, kind="ExternalInput")
    t_gidx = nc.dram_tensor("gidx", [128, total16], mybir.dt.int16,
                            kind="ExternalInput")
    t_dloc = nc.dram_tensor("dloc", [128, nch_tot], MM_DT, kind="ExternalInput")
    t_diso = nc.dram_tensor("diso", [128, NTILE], f32, kind="ExternalInput")
    t_gcnt = nc.dram_tensor("gcnt", [128, 2 * NTILE * NSEG], mybir.dt.int32,
                            kind="ExternalInput")
    t_out = nc.dram_tensor("out", [RPAD, N_CLS], f32, kind="ExternalOutput")

    # internal DRAM: per-half slabs and AllGathered tables
    g1slabA = nc.dram_tensor("g1slabA", [RA, H_GCN], MM_DT)
    g1slabB = nc.dram_tensor("g1slabB", [RB, H_GCN], MM_DT)
    g1tabA = nc.dram_tensor("g1tabA", [TA_ROWS, H_GCN], MM_DT, addr_space="Shared")
    g1tabB = nc.dram_tensor("g1tabB", [TB_ROWS, H_GCN], MM_DT, addr_space="Shared")
    g2slabA = nc.dram_tensor("g2slabA", [RA, C_PAD], MM_DT)
    g2slabB = nc.dram_tensor("g2slabB", [RB, C_PAD], MM_DT)
    g2tabA = nc.dram_tensor("g2tabA", [TA_ROWS, C_PAD], MM_DT, addr_space="Shared")
    g2tabB = nc.dram_tensor("g2tabB", [TB_ROWS, C_PAD], MM_DT, addr_space="Shared")
    acc_dC = nc.dram_tensor("accdC", [RPAD, H_GCN], MM_DT)
    acc_dE = nc.dram_tensor("accdE", [RPAD, CE], MM_DT)

    def seg_window(tabA, tabB, s):
        if s == 0:
            return tabA[0:SEGR_A, :]
        if s == 1:
            return tabA[SEGR_A:TA_ROWS, :]
        if s == 2:
            return tabB[0:SEGR_B, :]
        return tabB[SEGR_B:TB_ROWS, :]

    def allgather(slab, tab):
        nc.gpsimd.collective_compute(
            "AllGather", mybir.AluOpType.bypass,
            ins=[slab[:]], outs=[tab[:]],
            replica_groups=[list(range(NCORES))])

    with tile.TileContext(nc) as tc:
        with ExitStack() as octx:
            # constants, resident for the whole kernel
            cpool = octx.enter_context(tc.tile_pool(name="const", bufs=1))
            wmlp_sb = cpool.tile([128, 4 * H_MLP], MM_DT)
            for kc in range(4):
                nc.sync.dma_start(wmlp_sb[:, kc * H_MLP:(kc + 1) * H_MLP],
                                  t_wmlp[kc * 128:(kc + 1) * 128, :])
            w1_sb = cpool.tile([128, 2 * H_GCN], MM_DT)
            for kc in range(2):
                nc.sync.dma_start(w1_sb[:, kc * H_GCN:(kc + 1) * H_GCN],
                                  t_w1[kc * 128:(kc + 1) * 128, :])
            w2_sb = cpool.tile([128, 2 * CE], MM_DT)
            for kc in range(2):
                nc.sync.dma_start(w2_sb[:, kc * CE:(kc + 1) * CE],
                                  t_w2[kc * 128:(kc + 1) * 128, :])
            bmlp_sb = cpool.tile([128, 2], f32)
            nc.sync.dma_start(bmlp_sb[:], t_bmlp[:])
            b1_sb = cpool.tile([128, H_GCN], f32)
            nc.sync.dma_start(b1_sb[:], t_b1[:])
            b2_sb = cpool.tile([128, CE], f32)
            nc.sync.dma_start(b2_sb[:], t_b2[:])
            iota_sb = cpool.tile([128, CHMAX, 128], MM_DT)
            for k in range(CHMAX):
                nc.sync.dma_start(iota_sb[:, k, :], t_iota[:, k * 128:(k + 1) * 128])
            ident_sb = cpool.tile([128, 128], MM_DT)
            nc.sync.dma_start(ident_sb[:], t_ident[:])
            diso_sb = cpool.tile([128, NTILE], f32)
            nc.sync.dma_start(diso_sb[:], t_diso[:])
            gcnt_sb = cpool.tile([128, 2 * NTILE * NSEG], mybir.dt.int32)
            nc.sync.dma_start(gcnt_sb[:], t_gcnt[:])
            gidx_sb = cpool.tile([128, total16], mybir.dt.int16)
            for c0 in range(0, total16, 2048):
                c1 = min(c0 + 2048, total16)
                nc.sync.dma_start(gidx_sb[:, c0:c1], t_gidx[:, c0:c1])
            dloc_sb = cpool.tile([128, nch_tot], MM_DT)
            for c0 in range(0, nch_tot, 1024):
                c1 = min(c0 + 1024, nch_tot)
                nc.sync.dma_start(dloc_sb[:, c0:c1], t_dloc[:, c0:c1])

            def _body():
              # ---------------- Phase A: h = relu(x@Wmlp+b); g1' = dis*(h@W1)
              with ExitStack() as ctx:
                  xpool = ctx.enter_context(tc.tile_pool(name="xp", bufs=3))
                  hpool = ctx.enter_context(tc.tile_pool(name="hp", bufs=3))
                  opool = ctx.enter_context(tc.tile_pool(name="op", bufs=3))
                  ps_h = ctx.enter_context(tc.tile_pool(name="psh", bufs=2, space="PSUM"))
                  ps_g = ctx.enter_context(tc.tile_pool(name="psg", bufs=2, space="PSUM"))
                  ps_t = ctx.enter_context(tc.tile_pool(name="pst", bufs=3, space="PSUM"))
                  RT = 512
                  n_it = (RPAD + RT - 1) // RT
                  for it in range(n_it):
                      r0 = it * RT
                      rt = min(RT, RPAD - r0)
                      xt = xpool.tile([128, 4, RT], MM_DT, tag="xt")
                      for kc in range(4):
                          nc.sync.dma_start(xt[:, kc, :rt],
                                            t_xT[kc * 128:(kc + 1) * 128, r0:r0 + rt])
                      ht = hpool.tile([128, 2, RT], MM_DT, tag="ht")
                      for mh in range(2):
                          ph = ps_h.tile([128, RT], f32, tag="ph")
                          for kc in range(4):
                              nc.tensor.matmul(
                                  ph[:, :rt],
                                  wmlp_sb[:, kc * H_MLP + mh * 128:
                                          kc * H_MLP + (mh + 1) * 128],
                                  xt[:, kc, :rt],
                                  start=(kc == 0), stop=(kc == 3))
                          nc.scalar.activation(ht[:, mh, :rt], ph[:, :rt],
                                               mybir.ActivationFunctionType.Relu,
                                               bias=bmlp_sb[:, mh:mh + 1], scale=1.0)
                      g1t = hpool.tile([128, 2, RT], MM_DT, tag="g1t")
                      for mh in range(2):
                          pg = ps_g.tile([128, RT], f32, tag="pg")
                          for kc in range(2):
                              nc.tensor.matmul(
                                  pg[:, :rt],
                                  w1_sb[:, kc * H_GCN + mh * 128:
                                        kc * H_GCN + (mh + 1) * 128],
                                  ht[:, kc, :rt],
                                  start=(kc == 0), stop=(kc == 1))
                          nc.scalar.activation(g1t[:, mh, :rt], pg[:, :rt],
                                               mybir.ActivationFunctionType.Copy,
                                               bias=0.0, scale=1.0)
                      for rb in range(rt // 128):
                          tix = (r0 + rb * 128) // 128
                          pt = ps_t.tile([128, H_GCN], MM_DT, tag="pt")
                          for mh in range(2):
                              nc.tensor.transpose(
                                  pt[:, mh * 128:(mh + 1) * 128],
                                  g1t[:, mh, rb * 128:(rb + 1) * 128],
                                  ident_sb[:])
                          g1row = opool.tile([128, H_GCN], MM_DT, tag="g1row")
                          nc.vector.tensor_scalar_mul(g1row[:], pt[:],
                                                      diso_sb[:, tix:tix + 1])
                          rr = tix * 128
                          if rr < RA:
                              nc.sync.dma_start(g1slabA[rr:rr + 128, :], g1row[:])
                          else:
                              nc.sync.dma_start(g1slabB[rr - RA:rr - RA + 128, :],
                                                g1row[:])
                      # ---------- Phase B1: AllGather half A while computing B
                      if K_PHASES >= 2 and r0 + rt == RA:
                          allgather(g1slabA, g1tabA)

              # ---------------- Phase B2: AllGather half B
              if K_PHASES >= 2:
                  allgather(g1slabB, g1tabB)

              def aggregate_phase(ctx, tabA, tabB, elem, elemc, acc_d, epilogue, ph):
                  """Two-pass gather + one-hot matmul aggregation.

                  Pass 0 aggregates segments {0,1} (table A) into PSUM and
                  parks the partial in DRAM `acc_d` (HWDGE, off the busy
                  SWDGE queues); pass 1 aggregates segments {2,3}, merges
                  the reloaded partial back via an identity matmul, and
                  calls epilogue(t, pa)."""
                  gq = [0]
                  rcnt = nc.gpsimd.alloc_register(f"gcnt_r{ph}_{nc.next_id()}")
                  mpool = ctx.enter_context(tc.tile_pool(name="msgs", bufs=MBUFS))
                  ohpool = ctx.enter_context(tc.tile_pool(
                      name="oh", bufs=(3 if MERGEOH else 4)))
                  apool = ctx.enter_context(tc.tile_pool(name="apark", bufs=3))
                  ps_a = ctx.enter_context(tc.tile_pool(name="psa", bufs=4,
                                                        space="PSUM"))
                  # prime every msgs buffer once: 16-granular gathers leave
                  # the tail of the last 128-chunk as stale SBUF; the one-hot
                  # zeroes those edges (dloc=255) but NaN bit patterns from
                  # uninitialized SBUF would poison 0*NaN in the PE.
                  for _b in range(MBUFS):
                      for _s in range(2):
                          m = mpool.tile([128, cap, elem], MM_DT, tag=f"m{_s}")
                          nc.vector.memset(m[:], 0)
                  for p in range(2):
                      segs = (0, 1) if p == 0 else (2, 3)
                      for tg in range(NTG):
                          tiles = range(tg * TF, min(tg * TF + TF, NTILE))
                          t0 = tg * TF
                          msgs = {}
                          for s in segs:
                              gsz = int(N_tg[tg, s])
                              if gsz == 0:
                                  continue
                              off16 = int(off_ts[tg * TF, s]) // 16
                              m = mpool.tile([128, cap, elem], MM_DT,
                                             tag=f"m{s % 2}")
                              gcol = ph * NGC + t0 * NSEG + s
                              if SKIP_GATHER:
                                  nc.vector.memset(m[:, 0:1, 0:1], 0)
                              if not SKIP_GATHER:
                                  nc.gpsimd.reg_load(rcnt,
                                                     gcnt_sb[0:1, gcol:gcol + 1])
                                  nc.gpsimd.dma_gather(
                                      out_ap=m[:, :(gsz + 127) // 128, :],
                                      in_ap=seg_window(tabA, tabB, s),
                                      idxs_ap=gidx_sb[:, off16:off16 + gsz // 16],
                                      num_idxs=gsz, num_idxs_reg=rcnt,
                                      elem_size=elem,
                                      single_packet=KSP, queue_num=gq[0] % NQ)
                                  gq[0] += 1
                              msgs[s] = m
                          for t in tiles:
                              n_p = sum(int(CH[t, s]) for s in segs)
                              if SKIP_ONEHOT:
                                  n_p = 0
                              pa = ps_a.tile([128, elemc], f32, tag="pa")
                              alb = None
                              if p == 1:
                                  alb = apool.tile([128, elemc], MM_DT,
                                                   tag="aload")
                                  nc.sync.dma_start(
                                      alb[:], acc_d[t * 128:(t + 1) * 128, :])
                              ci = 0
                              for s in segs if not SKIP_ONEHOT else ():
                                  gts = int(G[t, s])
                                  if gts == 0:
                                      continue
                                  cglob = int(chunk_off[t, s])
                                  chs = int(CH[t, s])
                                  if MERGEOH:
                                      # one is_equal builds all chunks of this
                                      # (t, s): iota [128, chs, 128] vs dloc
                                      # column broadcast along the last dim.
                                      ohg = ohpool.tile([128, CHMAX, 128],
                                                        MM_DT, tag="oh")
                                      dv = dloc_sb[:, cglob:cglob + chs]
                                      bd = bass.AP(
                                          dv.tensor, dv.offset,
                                          [list(x) for x in dv.ap] + [[0, 128]])
                                      nc.vector.tensor_tensor(
                                          ohg[:, 0:chs, :],
                                          iota_sb[:, 0:chs, :], bd,
                                          mybir.AluOpType.is_equal)
                                      for k in range(chs):
                                          last = (ci == n_p - 1)
                                          nc.tensor.matmul(
                                              pa[:], ohg[:, k, :],
                                              msgs[s][:, k, 0:elemc],
                                              start=(ci == 0),
                                              stop=(last if p == 0 else False))
                                          ci += 1
                                  else:
                                      # fallback: per-chunk is_equal via the
                                      # same broadcast tensor_tensor
                                      for k in range(chs):
                                          oh = ohpool.tile([128, 1, 128], MM_DT,
                                                           tag="oh")
                                          dv = dloc_sb[:, cglob + k:cglob + k + 1]
                                          bd = bass.AP(
                                              dv.tensor, dv.offset,
                                              [list(x) for x in dv.ap] + [[0, 128]])
                                          nc.vector.tensor_tensor(
                                              oh[:, :, :],
                                              iota_sb[:, k:k + 1, :], bd,
                                              mybir.AluOpType.is_equal)
                                          last = (ci == n_p - 1)
                                          nc.tensor.matmul(
                                              pa[:], oh[:, 0, :],
                                              msgs[s][:, k, 0:elemc],
                                              start=(ci == 0),
                                              stop=(last if p == 0 else False))
                                          ci += 1
                              if p == 0:
                                  asb = apool.tile([128, elemc], MM_DT,
                                                   tag="apark")
                                  if n_p == 0:
                                      nc.vector.memset(asb[:], 0)
                                  else:
                                      nc.scalar.activation(
                                          asb[:], pa[:],
                                          mybir.ActivationFunctionType.Copy,
                                          bias=0.0, scale=1.0)
                                  nc.sync.dma_start(
                                      acc_d[t * 128:(t + 1) * 128, :], asb[:])
                              else:
                                  nc.tensor.matmul(pa[:], ident_sb[:],
                                                   alb[:],
                                                   start=(n_p == 0), stop=True)
                                  epilogue(t, pa)

              # ---------------- Phase C: L1 aggregate + h1 + g2'
              if K_PHASES >= 3:
                with ExitStack() as ctx:
                  hpool = ctx.enter_context(tc.tile_pool(name="h1p", bufs=3))
                  ps_t = ctx.enter_context(tc.tile_pool(name="pst2", bufs=2, space="PSUM"))
                  ps_2 = ctx.enter_context(tc.tile_pool(name="ps2", bufs=2, space="PSUM"))

                  def epi_c(t, pa):
                      h1 = hpool.tile([128, H_GCN], MM_DT, tag="h1")
                      if b1_nonzero:
                          nc.vector.tensor_scalar_mul(h1[:], pa[:],
                                                      diso_sb[:, t:t + 1])
                          nc.vector.tensor_add(h1[:], h1[:], b1_sb[:])
                          nc.scalar.activation(h1[:], h1[:],
                                               mybir.ActivationFunctionType.Relu,
                                               bias=0.0, scale=1.0)
                      else:
                          nc.scalar.activation(h1[:], pa[:],
                                               mybir.ActivationFunctionType.Relu,
                                               bias=0.0, scale=diso_sb[:, t:t + 1])
                      # g2 = dis * (h1 @ W2)
                      pt = ps_t.tile([128, 2, 128], MM_DT, tag="ptc")
                      h1t = hpool.tile([128, 2, 128], MM_DT, tag="h1t")
                      p2 = ps_2.tile([128, CE], f32, tag="p2")
                      for kk in range(2):
                          nc.tensor.transpose(pt[:, kk, :],
                                              h1[:, kk * 128:(kk + 1) * 128],
                                              ident_sb[:])
                          nc.scalar.activation(h1t[:, kk, :], pt[:, kk, :],
                                               mybir.ActivationFunctionType.Copy,
                                               bias=0.0, scale=1.0)
                      for kk in range(2):
                          nc.tensor.matmul(p2[:], h1t[:, kk, :],
                                           w2_sb[:, kk * CE:(kk + 1) * CE],
                                           start=(kk == 0), stop=(kk == 1))
                      g2row = hpool.tile([128, CE], MM_DT, tag="g2row")
                      nc.scalar.activation(g2row[:], p2[:],
                                           mybir.ActivationFunctionType.Copy,
                                           bias=0.0, scale=diso_sb[:, t:t + 1])
                      rr = t * 128
                      if rr < RA:
                          nc.sync.dma_start(g2slabA[rr:rr + 128, 0:CE], g2row[:])
                      else:
                          nc.sync.dma_start(g2slabB[rr - RA:rr - RA + 128, 0:CE],
                                            g2row[:])
                      # ---------- Phase D1: AllGather half A of g2'
                      if K_PHASES >= 4 and t == TILES_A - 1:
                          allgather(g2slabA, g2tabA)

                  aggregate_phase(ctx, g1tabA, g1tabB, H_GCN, H_GCN, acc_dC, epi_c, 0)

              # ---------------- Phase D2: AllGather half B of g2'
              if K_PHASES >= 4:
                  allgather(g2slabB, g2tabB)

              # ---------------- Phase E: L2 aggregate -> out
              if K_PHASES >= 5:
                with ExitStack() as ctx:
                  hpool = ctx.enter_context(tc.tile_pool(name="outp", bufs=3))

                  def epi_e(t, pa):
                      ot = hpool.tile([128, CE], f32, tag="ot")
                      nc.scalar.activation(ot[:], pa[:],
                                           mybir.ActivationFunctionType.Copy,
                                           bias=0.0, scale=diso_sb[:, t:t + 1])
                      ot2 = hpool.tile([128, CE], f32, tag="ot2")
                      nc.vector.tensor_add(ot2[:], ot[:], b2_sb[:])
                      nc.sync.dma_start(t_out[t * 128:(t + 1) * 128, :],
                                        ot2[:, :N_CLS])

                  aggregate_phase(ctx, g2tabA, g2tabB, C_PAD, CE, acc_dE, epi_e, 1)

            for _rep in range(reps):
                _body()

    nc.compile()
    _split_multi_waits(nc)
    return nc, N_tg


def _split_multi_waits(nc, max_waits=1):
    """walrus CoreV3 rejects >max_waits sem waits on one instruction; split
    extras onto preceding NOPs on the same engine."""
    n = 0
    for fn in nc.m.functions:
        for bb in fn.blocks:
            insts = bb.instructions
            i = 0
            while i < len(insts):
                inst = insts[i]
                si = inst.sync_info
                if si is not None and si.on_wait and len(si.on_wait) > max_waits:
                    waits = list(si.on_wait)
                    keep = waits[-max_waits:]
                    extra = waits[:-max_waits]
                    new_insts = []
                    for cs in range(0, len(extra), max_waits):
                        nop = mybir.InstNoOp(
                            name=f"I-waitsplit-{id(inst)}-{cs}-{n}",
                            sync_info=mybir.SyncInfo(
                                on_wait=extra[cs:cs + max_waits], on_update=[]),
                            bass_nofuse=True,
                            engine=inst.engine)
                        new_insts.append(nop)
                        n += 1
                    si.on_wait = keep
                    for j, nop in enumerate(new_insts):
                        insts.insert(i + j, nop)
                    i += len(new_insts)
                i += 1
    return n


def prepare(x, edge_index, W_mlp, b_mlp, W1, b1, W2, b2, reps=None, k_phases=None):
    x = np.asarray(x, dtype=np.float32)
    W_mlp_ = np.asarray(W_mlp, dtype=np.float32)
    b_mlp_ = np.asarray(b_mlp, dtype=np.float32)
    W1_ = np.asarray(W1, dtype=np.float32)
    b1_ = np.asarray(b1, dtype=np.float32)
    W2_ = np.asarray(W2, dtype=np.float32)
    b2_ = np.asarray(b2, dtype=np.float32)

    if reps is None:
        reps = int(os.environ.get("KREPS", "1"))
    G, off_ts, chunk_off, total, gidx, dloc, diso, cntc = _preprocess(edge_index)
    nc, _ = _build_program(G, off_ts, chunk_off, total,
                           b1_nonzero=bool(np.abs(b1_).max() > 0), reps=reps,
                           k_phases=k_phases)

    W2p = np.zeros((H_GCN, CE), np.float32)
    W2p[:, :N_CLS] = W2_
    b2p = np.zeros(CE, np.float32)
    b2p[:N_CLS] = b2_
    bmlp_pk = np.ascontiguousarray(b_mlp_.reshape(2, 128).T)
    b1bc = np.tile(b1_[None, :], (128, 1)).astype(np.float32)
    b2bc = np.tile(b2p[None, :], (128, 1)).astype(np.float32)
    chmax = int(((G + 127) // 128).max())
    iota = np.tile(np.arange(128, dtype=np.float32)[None, :],
                   (128, chmax)).astype(MM_NP)
    ident = np.eye(128, dtype=MM_NP)

    in_maps = []
    for c in range(NCORES):
        xs = np.zeros((RPAD, N_FEAT), np.float32)
        xs[:RPC] = x[c * RPC:(c + 1) * RPC]
        in_maps.append({
            "xT": np.ascontiguousarray(xs.T).astype(MM_NP),
            "wmlp": W_mlp_.astype(MM_NP), "w1": W1_.astype(MM_NP),
            "w2": W2p.astype(MM_NP),
            "bmlp": bmlp_pk, "b1bc": b1bc, "b2bc": b2bc,
            "iota": iota, "ident": ident,
            "gidx": gidx[c], "dloc": dloc[c].astype(MM_NP), "diso": diso[c],
            "gcnt": np.broadcast_to(
                np.concatenate([cntc[c].reshape(-1), cntc[c].reshape(-1)])
                .astype(np.int32)[None, :], (128, 2 * NTILE * NSEG)).copy(),
        })

    return nc, in_maps


def kernel(**inputs):
    nc, in_maps = prepare(**inputs)
    res = run_bass_kernel_spmd(nc, in_maps, list(range(NCORES)))
    global last_results
    last_results = res
    out = np.concatenate(
        [res.results[c]["out"][:RPC] for c in range(NCORES)], axis=0)
    return out.astype(np.float32)


last_results = None


if __name__ == "__main__":
    import reference
    from np_ref import np_reference
    inputs = {k: np.asarray(v) for k, v in reference.setup_inputs().items()}
    got = kernel(**inputs)
    exp = np_reference(**inputs)
    denom = np.abs(exp).max()
    err = np.abs(got - exp).max()
    print(f"abs err {err}  rel err {err / denom}  scale {denom}")



# revision 33
# speedup vs baseline: 1.0072x; 1.0072x over previous
"""GCN (MLP pre-encoder + 2 GCNConv layers) on 8 Trainium2 NeuronCores.

Strategy (graph/data parallel, per sharding hint):
- Nodes sharded by rows across 8 cores (12500 -> padded 12544 each).
- Edges partitioned by destination core, sorted by (dest tile-group,
  source segment, dest tile); self-loops appended as ordinary edges.
- Symmetric normalization folded into the gathered feature tables:
  g' = dis * g, so each message is an unweighted row gather and the
  per-dest scale dis[d] is applied once per output row.
- Aggregation per 128-dest tile: dma_gather rows of the AllGathered
  table, scatter-add within the tile via a one-hot matmul on the PE
  (onehot[m, d] = (dest_local[m] == d)), accumulated in PSUM.
- v2: phase A runs in bf16; gathers merged across TF dest tiles per
  instruction; PSUM->SBUF copies and scaling on the Activation engine.
- v3: each AllGather is split into two half-node-range collectives
  (tables A/B) so communication overlaps compute: AG(A-half) runs
  while phase A computes the B half; aggregation runs in two passes
  (sources in half A, then half B), with the pass-1 partial parked in
  SBUF and merged back into PSUM via an identity matmul, so pass 1
  overlaps the second collective.
- v4 (5.8ms -> ~2.3ms): the SWDGE descriptor ring was the choke: at
  the default 16KB scratch (1024 descs) a ~1131-desc gather cannot
  even fit, so each gather's ring-space wait serialized desc-gen
  behind the FULL drain of every prior same-queue gather, and the DMA
  stream never overlapped compute.  dynamic_dma_scratch_size=49152
  (3072 descs) keeps 2-3 gathers in flight per queue.  One-hot
  builds are merged (one broadcast tensor_tensor is_equal per (tile,
  seg) builds all its 128-edge chunks; dloc stored bf16) cutting DVE
  instructions ~9x.  The pass-0 partial is parked in DRAM via the
  idle HWDGE queues instead of a 50KB SBUF slab, freeing SBUF for
  msgs double-buffering (6 bufs).  Phase E computes at CE=64 columns
  (N_CLS=40) while gather rows stay at the 256B minimum.  Gathers are
  per-descriptor latency-bound (~6.5ns/desc/queue, 4 queues), so the
  ~1.35ms two-phase gather stream is the roofline; compute hides
  under it almost completely.
"""
import os
import sys
sys.path.insert(0, "/opt/trn_rl_repo")
import numpy as np
import ml_dtypes
import concourse.bass as bass
import concourse.bacc as bacc
import concourse.mybir as mybir
from concourse import tile
from concourse.bass_utils import run_bass_kernel_spmd
from contextlib import ExitStack

N_NODES = 100000
N_FEAT = 512
H_MLP = 256
H_GCN = 256
N_CLS = 40
NCORES = 8
RPC = 12500         # real rows per core
RPAD = 12544        # padded rows per core (98 * 128)
NTILE = 98          # dest tiles per core
NSEG = 4
TF = int(os.environ.get("KTF", "1"))  # dest tiles merged per gather instruction
NTG = (NTILE + TF - 1) // TF
# half split: rows [0, RA) -> table A, rows [RA, RPC) -> table B
RA = 6144           # 48 dest tiles (phase-A RT=512 boundary: 12 iters)
RB = RPAD - RA      # 6400 rows, 50 tiles
TILES_A = RA // 128
TA_ROWS = NCORES * RA    # 49152 rows in table A
TB_ROWS = NCORES * RB    # 51200 rows in table B
SEGR_A = TA_ROWS // 2    # 24576 (< 2**15)
SEGR_B = TB_ROWS // 2    # 25600 (< 2**15)
MM_DT = mybir.dt.bfloat16
MM_NP = ml_dtypes.bfloat16
K_PHASES = int(os.environ.get("K_PHASES", "5"))  # debug: stop after N phases
NQ = int(os.environ.get("KQUEUES", "4"))        # SWDGE queues for gathers
KSP = os.environ.get("KSP", "0") == "1"         # single_packet mode
MBUFS = int(os.environ.get("KMBUFS", "6"))      # msgs tile-pool buffers
# SWDGE descriptor-ring scratch: 49152B = 3072 descs, so 2-3 gathers
# (~1131 descs each) can be in flight per queue instead of 0-1 at the
# 16384B default (ring-space wait serialized desc-gen behind the full
# drain of every previous same-queue gather).
KSCRATCH = int(os.environ.get("KSCRATCH", "49152"))
MERGEOH = os.environ.get("KMERGEOH", "1") == "1"  # one-hot build per (t,s)
SKIP_GATHER = os.environ.get("KSKIP_GATHER", "0") == "1"   # timing ablation
SKIP_ONEHOT = os.environ.get("KSKIP_ONEHOT", "0") == "1"   # timing ablation
C_PAD = 128         # N_CLS padded so gather rows are 256B
CE = 64             # phase-E compute width (N_CLS=40 padded to 64)


def _preprocess(edge_index):
    """Host-side graph preprocessing -> per-core edge structures."""
    row = np.asarray(edge_index[0], dtype=np.int64)
    col = np.asarray(edge_index[1], dtype=np.int64)
    deg = np.bincount(col, minlength=N_NODES).astype(np.float32) + 1.0
    dis = (1.0 / np.sqrt(deg)).astype(np.float32)

    allr = np.arange(N_NODES, dtype=np.int64)
    dest = np.concatenate([row, allr])
    src = np.concatenate([col, allr])

    core = dest // RPC
    dlocal = dest % RPC
    tile_id = dlocal // 128
    dl = (dlocal % 128).astype(np.float32)

    # source -> (half table, row, segment, int16 offset)
    sc = src // RPC
    sr = src % RPC
    h = (sr >= RA).astype(np.int64)
    trow = np.where(h == 0, sc * RA + sr, sc * RB + (sr - RA))
    segr = np.where(h == 0, SEGR_A, SEGR_B)
    seg = 2 * h + (trow >= segr)
    sloc = (trow - (trow >= segr) * segr).astype(np.int16)

    # per-(tile, seg) group sizes, padded to 16 (gather descriptors are
    # per-row and latency-bound, so padding is pure waste; the one-hot
    # masks the <=15 pad edges and the stale tail of the last 128-chunk),
    # shared across cores (SPMD)
    ngrp = NTILE * NSEG
    key_ts = (core * ngrp + tile_id * NSEG + seg).astype(np.int64)
    cnt = np.bincount(key_ts, minlength=NCORES * ngrp).reshape(NCORES, NTILE, NSEG)
    G = cnt.max(axis=0)
    G = ((G + 15) // 16) * 16             # [NTILE, NSEG]

    # storage order: (tg, s, t-within-tg); group offsets in that order
    order_keys = []
    for tg in range(NTG):
        tiles = range(tg * TF, min(tg * TF + TF, NTILE))
        for s in range(NSEG):
            for t in tiles:
                order_keys.append((t, s))
    sizes = np.array([G[t, s] for (t, s) in order_keys], np.int64)
    offs = np.concatenate([[0], np.cumsum(sizes)])[:-1]
    off_ts = np.zeros((NTILE, NSEG), np.int64)
    for (t, s), o in zip(order_keys, offs):
        off_ts[t, s] = o
    total = int(sizes.sum())

    # rank of each edge within its (core, t, s) group
    order = np.argsort(key_ts, kind="stable")
    sk = key_ts[order]
    starts = np.searchsorted(sk, np.arange(NCORES * ngrp))
    rank = np.arange(len(sk)) - starts[sk]
    ckey = sk // ngrp
    tkey = (sk % ngrp) // NSEG
    skey = sk % NSEG
    pos = off_ts[tkey, skey] + rank

    idx_arr = np.full((NCORES, total), -1, np.int16)   # pad -> skipped
    idx_arr[ckey, pos] = sloc[order]
    gidx = [np.tile(idx_arr[c].reshape(-1, 16).T, (8, 1)).copy() for c in range(NCORES)]

    # dest-in-tile table in a separate 128-chunk-padded layout (group offsets
    # are only 16-aligned; one-hot columns must be 128-edge chunks)
    CH = (G + 127) // 128                  # chunks per (t, s)
    chunk_off = np.zeros((NTILE, NSEG), np.int64)
    o = 0
    for (t, s) in order_keys:
        chunk_off[t, s] = o
        o += int(CH[t, s])
    total_ch = o
    dl_arr = np.full((NCORES, total_ch * 128), 255.0, np.float32)
    posc = chunk_off[tkey, skey] * 128 + rank
    dl_arr[ckey, posc] = dl[order]
    dloc = [np.ascontiguousarray(dl_arr[c].reshape(-1, 128).T) for c in range(NCORES)]

    dis_pad = np.ones(NCORES * RPAD, np.float32)
    for c in range(NCORES):
        dis_pad[c * RPAD:c * RPAD + RPC] = dis[c * RPC:(c + 1) * RPC]
    diso = [np.ascontiguousarray(
        dis_pad[c * RPAD:(c + 1) * RPAD].reshape(NTILE, 128).T) for c in range(NCORES)]

    return G, off_ts, chunk_off, total, gidx, dloc, diso, cnt


def _build_program(G, off_ts, chunk_off, total, b1_nonzero=False, reps=1,
                   k_phases=None):
    NGC = NTILE * NSEG  # gather-count columns per aggregation phase
    """Build the SPMD Bass program (identical across cores)."""
    K_PHASES = globals()["K_PHASES"] if k_phases is None else k_phases
    total16 = total // 16
    CH = (G + 127) // 128                  # chunks per (t, s)
    nch_tot = int(CH.sum())
    CHMAX = int(CH.max())
    N_tg = np.zeros((NTG, NSEG), np.int64)
    for tg in range(NTG):
        tiles = range(tg * TF, min(tg * TF + TF, NTILE))
        for s in range(NSEG):
            N_tg[tg, s] = sum(int(G[t, s]) for t in tiles)
    cap = max(1, int(((N_tg + 127) // 128).max()))   # msgs capacity (chunks)

    f32 = mybir.dt.float32
    nc = bacc.Bacc("TRN2", target_bir_lowering=False, debug=False,
                   num_devices=NCORES, num_swdge_queues=NQ,
                   dynamic_dma_scratch_size=KSCRATCH)

    # inputs
    t_xT = nc.dram_tensor("xT", [N_FEAT, RPAD], MM_DT, kind="ExternalInput")
    t_wmlp = nc.dram_tensor("wmlp", [N_FEAT, H_MLP], MM_DT, kind="ExternalInput")
    t_w1 = nc.dram_tensor("w1", [H_MLP, H_GCN], MM_DT, kind="ExternalInput")
    t_w2 = nc.dram_tensor("w2", [H_GCN, CE], MM_DT, kind="ExternalInput")
    t_bmlp = nc.dram_tensor("bmlp", [128, 2], f32, kind="ExternalInput")
    t_b1 = nc.dram_tensor("b1bc", [128, H_GCN], f32, kind="ExternalInput")
    t_b2 = nc.dram_tensor("b2bc", [128, CE], f32, kind="ExternalInput")
    t_iota = nc.dram_tensor("iota", [128, CHMAX * 128], MM_DT,
                            kind="ExternalInput")
    t_ident = nc.dram_tensor("ident", [128, 128], MM_DT, kind="ExternalInput")
    t_gidx = nc.dram_tensor("gidx", [128, total16], mybir.dt.int16,
                            kind="ExternalInput")
    t_dloc = nc.dram_tensor("dloc", [128, nch_tot], MM_DT, kind="ExternalInput")
    t_diso = nc.dram_tensor("diso", [128, NTILE], f32, kind="ExternalInput")
    t_gcnt = nc.dram_tensor("gcnt", [128, 2 * NTILE * NSEG], mybir.dt.int32,
                            kind="ExternalInput")
    t_out = nc.dram_tensor("out", [RPAD, N_CLS], f32, kind="ExternalOutput")

    # internal DRAM: per-half slabs and AllGathered tables
    g1slabA = nc.dram_tensor("g1slabA", [RA, H_GCN], MM_DT)
    g1slabB = nc.dram_tensor("g1slabB", [RB, H_GCN], MM_DT)
    g1tabA = nc.dram_tensor("g1tabA", [TA_ROWS, H_GCN], MM_DT, addr_space="Shared")
    g1tabB = nc.dram_tensor("g1tabB", [TB_ROWS, H_GCN], MM_DT, addr_space="Shared")
    g2slabA = nc.dram_tensor("g2slabA", [RA, C_PAD], MM_DT)
    g2slabB = nc.dram_tensor("g2slabB", [RB, C_PAD], MM_DT)
    g2tabA = nc.dram_tensor("g2tabA", [TA_ROWS, C_PAD], MM_DT, addr_space="Shared")
    g2tabB = nc.dram_tensor("g2tabB", [TB_ROWS, C_PAD], MM_DT, addr_space="Shared")
    acc_dC = nc.dram_tensor("accdC", [RPAD, H_GCN], MM_DT)
    acc_dE = nc.dram_tensor("accdE", [RPAD, CE], MM_DT)

    def seg_window(tabA, tabB, s):
        if s == 0:
            return tabA[0:SEGR_A, :]
        if s == 1:
            return tabA[SEGR_A:TA_ROWS, :]
        if s == 2:
            return tabB[0:SEGR_B, :]
        return tabB[SEGR_B:TB_ROWS, :]

    def allgather(slab, tab):
        nc.gpsimd.collective_compute(
            "AllGather", mybir.AluOpType.bypass,
            ins=[slab[:]], outs=[tab[:]],
            replica_groups=[list(range(NCORES))])

    with tile.TileContext(nc) as tc:
        with ExitStack() as octx:
            # constants, resident for the whole kernel
            cpool = octx.enter_context(tc.tile_pool(name="const", bufs=1))
            wmlp_sb = cpool.tile([128, 4 * H_MLP], MM_DT)
            for kc in range(4):
                nc.sync.dma_start(wmlp_sb[:, kc * H_MLP:(kc + 1) * H_MLP],
                                  t_wmlp[kc * 128:(kc + 1) * 128, :])
            w1_sb = cpool.tile([128, 2 * H_GCN], MM_DT)
            for kc in range(2):
                nc.sync.dma_start(w1_sb[:, kc * H_GCN:(kc + 1) * H_GCN],
                                  t_w1[kc * 128:(kc + 1) * 128, :])
            w2_sb = cpool.tile([128, 2 * CE], MM_DT)
            for kc in range(2):
                nc.sync.dma_start(w2_sb[:, kc * CE:(kc + 1) * CE],
                                  t_w2[kc * 128:(kc + 1) * 128, :])
            bmlp_sb = cpool.tile([128, 2], f32)
            nc.sync.dma_start(bmlp_sb[:], t_bmlp[:])
            b1_sb = cpool.tile([128, H_GCN], f32)
            nc.sync.dma_start(b1_sb[:], t_b1[:])
            b2_sb = cpool.tile([128, CE], f32)
            nc.sync.dma_start(b2_sb[:], t_b2[:])
            iota_sb = cpool.tile([128, CHMAX, 128], MM_DT)
            for k in range(CHMAX):
                nc.sync.dma_start(iota_sb[:, k, :], t_iota[:, k * 128:(k + 1) * 128])
            ident_sb = cpool.tile([128, 128], MM_DT)
            nc.sync.dma_start(ident_sb[:], t_ident[:])
            diso_sb = cpool.tile([128, NTILE], f32)
            nc.sync.dma_start(diso_sb[:], t_diso[:])
            gcnt_sb = cpool.tile([128, 2 * NTILE * NSEG], mybir.dt.int32)
            nc.sync.dma_start(gcnt_sb[:], t_gcnt[:])
            gidx_sb = cpool.tile([128, total16], mybir.dt.int16)
            for c0 in range(0, total16, 2048):
                c1 = min(c0 + 2048, total16)
                nc.sync.dma_start(gidx_sb[:, c0:c1], t_gidx[:, c0:c1])
            dloc_sb = cpool.tile([128, nch_tot], MM_DT)
            for c0 in range(0, nch_tot, 1024):
                c1 = min(c0 + 1024, nch_tot)
                nc.sync.dma_start(dloc_sb[:, c0:c1], t_dloc[:, c0:c1])

            def _body():
              # ---------------- Phase A: h = relu(x@Wmlp+b); g1' = dis*(h@W1)
              with ExitStack() as ctx:
                  xpool = ctx.enter_context(tc.tile_pool(name="xp", bufs=3))
                  hpool = ctx.enter_context(tc.tile_pool(name="hp", bufs=3))
                  opool = ctx.enter_context(tc.tile_pool(name="op", bufs=3))
                  ps_h = ctx.enter_context(tc.tile_pool(name="psh", bufs=2, space="PSUM"))
                  ps_g = ctx.enter_context(tc.tile_pool(name="psg", bufs=2, space="PSUM"))
                  ps_t = ctx.enter_context(tc.tile_pool(name="pst", bufs=3, space="PSUM"))
                  RT = 512
                  n_it = (RPAD + RT - 1) // RT
                  for it in range(n_it):
                      r0 = it * RT
                      rt = min(RT, RPAD - r0)
                      xt = xpool.tile([128, 4, RT], MM_DT, tag="xt")
                      for kc in range(4):
                          nc.sync.dma_start(xt[:, kc, :rt],
                                            t_xT[kc * 128:(kc + 1) * 128, r0:r0 + rt])
                      ht = hpool.tile([128, 2, RT], MM_DT, tag="ht")
                      for mh in range(2):
                          ph = ps_h.tile([128, RT], f32, tag="ph")
                          for kc in range(4):
                              nc.tensor.matmul(
                                  ph[:, :rt],
                                  wmlp_sb[:, kc * H_MLP + mh * 128:
                                          kc * H_MLP + (mh + 1) * 128],
                                  xt[:, kc, :rt],
                                  start=(kc == 0), stop=(kc == 3))
                          nc.scalar.activation(ht[:, mh, :rt], ph[:, :rt],
                                               mybir.ActivationFunctionType.Relu,
                                               bias=bmlp_sb[:, mh:mh + 1], scale=1.0)
                      g1t = hpool.tile([128, 2, RT], MM_DT, tag="g1t")
                      for mh in range(2):
                          pg = ps_g.tile([128, RT], f32, tag="pg")
                          for kc in range(2):
                              nc.tensor.matmul(
                                  pg[:, :rt],
                                  w1_sb[:, kc * H_GCN + mh * 128:
                                        kc * H_GCN + (mh + 1) * 128],
                                  ht[:, kc, :rt],
                                  start=(kc == 0), stop=(kc == 1))
                          nc.scalar.activation(g1t[:, mh, :rt], pg[:, :rt],
                                               mybir.ActivationFunctionType.Copy,
                                               bias=0.0, scale=1.0)
                      for rb in range(rt // 128):
                          tix = (r0 + rb * 128) // 128
                          pt = ps_t.tile([128, H_GCN], MM_DT, tag="pt")
                          for mh in range(2):
                              nc.tensor.transpose(
                                  pt[:, mh * 128:(mh + 1) * 128],
                                  g1t[:, mh, rb * 128:(rb + 1) * 128],
                                  ident_sb[:])
                          g1row = opool.tile([128, H_GCN], MM_DT, tag="g1row")
                          nc.vector.tensor_scalar_mul(g1row[:], pt[:],
                                                      diso_sb[:, tix:tix + 1])
                          rr = tix * 128
                          if rr < RA:
                              nc.sync.dma_start(g1slabA[rr:rr + 128, :], g1row[:])
                          else:
                              nc.sync.dma_start(g1slabB[rr - RA:rr - RA + 128, :],
                                                g1row[:])
                      # ---------- Phase B1: AllGather half A while computing B
                      if K_PHASES >= 2 and r0 + rt == RA:
                          allgather(g1slabA, g1tabA)

              # ---------------- Phase B2: AllGather half B
              if K_PHASES >= 2:
                  allgather(g1slabB, g1tabB)

              def aggregate_phase(ctx, tabA, tabB, elem, elemc, acc_d, epilogue, ph):
                  """Two-pass gather + one-hot matmul aggregation.

                  Pass 0 aggregates segments {0,1} (table A) into PSUM and
                  parks the partial in DRAM `acc_d` (HWDGE, off the busy
                  SWDGE queues); pass 1 aggregates segments {2,3}, merges
                  the reloaded partial back via an identity matmul, and
                  calls epilogue(t, pa)."""
                  gq = [0]
                  rcnt = nc.gpsimd.alloc_register(f"gcnt_r{ph}_{nc.next_id()}")
                  mpool = ctx.enter_context(tc.tile_pool(name="msgs", bufs=MBUFS))
                  ohpool = ctx.enter_context(tc.tile_pool(
                      name="oh", bufs=(3 if MERGEOH else 4)))
                  apool = ctx.enter_context(tc.tile_pool(name="apark", bufs=3))
                  ps_a = ctx.enter_context(tc.tile_pool(name="psa", bufs=3,
                                                        space="PSUM"))
                  # prime every msgs buffer once: 16-granular gathers leave
                  # the tail of the last 128-chunk as stale SBUF; the one-hot
                  # zeroes those edges (dloc=255) but NaN bit patterns from
                  # uninitialized SBUF would poison 0*NaN in the PE.
                  for _b in range(MBUFS):
                      for _s in range(2):
                          m = mpool.tile([128, cap, elem], MM_DT, tag=f"m{_s}")
                          nc.vector.memset(m[:], 0)
                  for p in range(2):
                      segs = (0, 1) if p == 0 else (2, 3)
                      for tg in range(NTG):
                          tiles = range(tg * TF, min(tg * TF + TF, NTILE))
                          t0 = tg * TF
                          msgs = {}
                          for s in segs:
                              gsz = int(N_tg[tg, s])
                              if gsz == 0:
                                  continue
                              off16 = int(off_ts[tg * TF, s]) // 16
                              m = mpool.tile([128, cap, elem], MM_DT,
                                             tag=f"m{s % 2}")
                              gcol = ph * NGC + t0 * NSEG + s
                              if SKIP_GATHER:
                                  nc.vector.memset(m[:, 0:1, 0:1], 0)
                              if not SKIP_GATHER:
                                  nc.gpsimd.reg_load(rcnt,
                                                     gcnt_sb[0:1, gcol:gcol + 1])
                                  nc.gpsimd.dma_gather(
                                      out_ap=m[:, :(gsz + 127) // 128, :],
                                      in_ap=seg_window(tabA, tabB, s),
                                      idxs_ap=gidx_sb[:, off16:off16 + gsz // 16],
                                      num_idxs=gsz, num_idxs_reg=rcnt,
                                      elem_size=elem,
                                      single_packet=KSP, queue_num=gq[0] % NQ)
                                  gq[0] += 1
                              msgs[s] = m
                          for t in tiles:
                              n_p = sum(int(CH[t, s]) for s in segs)
                              if SKIP_ONEHOT:
                                  n_p = 0
                              pa = ps_a.tile([128, elemc], f32, tag="pa")
                              alb = None
                              if p == 1:
                                  alb = apool.tile([128, elemc], MM_DT,
                                                   tag="aload")
                                  nc.sync.dma_start(
                                      alb[:], acc_d[t * 128:(t + 1) * 128, :])
                              ci = 0
                              for s in segs if not SKIP_ONEHOT else ():
                                  gts = int(G[t, s])
                                  if gts == 0:
                                      continue
                                  cglob = int(chunk_off[t, s])
                                  chs = int(CH[t, s])
                                  if MERGEOH:
                                      # one is_equal builds all chunks of this
                                      # (t, s): iota [128, chs, 128] vs dloc
                                      # column broadcast along the last dim.
                                      ohg = ohpool.tile([128, CHMAX, 128],
                                                        MM_DT, tag="oh")
                                      dv = dloc_sb[:, cglob:cglob + chs]
                                      bd = bass.AP(
                                          dv.tensor, dv.offset,
                                          [list(x) for x in dv.ap] + [[0, 128]])
                                      nc.vector.tensor_tensor(
                                          ohg[:, 0:chs, :],
                                          iota_sb[:, 0:chs, :], bd,
                                          mybir.AluOpType.is_equal)
                                      for k in range(chs):
                                          last = (ci == n_p - 1)
                                          nc.tensor.matmul(
                                              pa[:], ohg[:, k, :],
                                              msgs[s][:, k, 0:elemc],
                                              start=(ci == 0),
                                              stop=(last if p == 0 else False))
                                          ci += 1
                                  else:
                                      # fallback: per-chunk is_equal via the
                                      # same broadcast tensor_tensor
                                      for k in range(chs):
                                          oh = ohpool.tile([128, 1, 128], MM_DT,
                                                           tag="oh")
                                          dv = dloc_sb[:, cglob + k:cglob + k + 1]
                                          bd = bass.AP(
                                              dv.tensor, dv.offset,
                                              [list(x) for x in dv.ap] + [[0, 128]])
                                          nc.vector.tensor_tensor(
                                              oh[:, :, :],
                                              iota_sb[:, k:k + 1, :], bd,
                                              mybir.AluOpType.is_equal)
                                          last = (ci == n_p - 1)
                                          nc.tensor.matmul(
                                              pa[:], oh[:, 0, :],
                                              msgs[s][:, k, 0:elemc],
                                              start=(ci == 0),
                                              stop=(last if p == 0 else False))
                                          ci += 1
                              if p == 0:
                                  asb = apool.tile([128, elemc], MM_DT,
                                                   tag="apark")
                                  if n_p == 0:
                                      nc.vector.memset(asb[:], 0)
                                  else:
                                      nc.scalar.activation(
                                          asb[:], pa[:],
                                          mybir.ActivationFunctionType.Copy,
                                          bias=0.0, scale=1.0)
                                  nc.sync.dma_start(
                                      acc_d[t * 128:(t + 1) * 128, :], asb[:])
                              else:
                                  nc.tensor.matmul(pa[:], ident_sb[:],
                                                   alb[:],
                                                   start=(n_p == 0), stop=True)
                                  epilogue(t, pa)

              # ---------------- Phase C: L1 aggregate + h1 + g2'
              if K_PHASES >= 3:
                with ExitStack() as ctx:
                  hpool = ctx.enter_context(tc.tile_pool(name="h1p", bufs=3))
                  ps_t = ctx.enter_context(tc.tile_pool(name="pst2", bufs=3, space="PSUM"))
                  ps_2 = ctx.enter_context(tc.tile_pool(name="ps2", bufs=2, space="PSUM"))

                  def epi_c(t, pa):
                      h1 = hpool.tile([128, H_GCN], MM_DT, tag="h1")
                      if b1_nonzero:
                          nc.vector.tensor_scalar_mul(h1[:], pa[:],
                                                      diso_sb[:, t:t + 1])
                          nc.vector.tensor_add(h1[:], h1[:], b1_sb[:])
                          nc.scalar.activation(h1[:], h1[:],
                                               mybir.ActivationFunctionType.Relu,
                                               bias=0.0, scale=1.0)
                      else:
                          nc.scalar.activation(h1[:], pa[:],
                                               mybir.ActivationFunctionType.Relu,
                                               bias=0.0, scale=diso_sb[:, t:t + 1])
                      # g2 = dis * (h1 @ W2)
                      pt = ps_t.tile([128, 2, 128], MM_DT, tag="ptc")
                      h1t = hpool.tile([128, 2, 128], MM_DT, tag="h1t")
                      p2 = ps_2.tile([128, CE], f32, tag="p2")
                      for kk in range(2):
                          nc.tensor.transpose(pt[:, kk, :],
                                              h1[:, kk * 128:(kk + 1) * 128],
                                              ident_sb[:])
                          nc.scalar.activation(h1t[:, kk, :], pt[:, kk, :],
                                               mybir.ActivationFunctionType.Copy,
                                               bias=0.0, scale=1.0)
                      for kk in range(2):
                          nc.tensor.matmul(p2[:], h1t[:, kk, :],
                                           w2_sb[:, kk * CE:(kk + 1) * CE],
                                           start=(kk == 0), stop=(kk == 1))
                      g2row = hpool.tile([128, CE], MM_DT, tag="g2row")
                      nc.scalar.activation(g2row[:], p2[:],
                                           mybir.ActivationFunctionType.Copy,
                                           bias=0.0, scale=diso_sb[:, t:t + 1])
                      rr = t * 128
                      if rr < RA:
                          nc.sync.dma_start(g2slabA[rr:rr + 128, 0:CE], g2row[:])
                      else:
                          nc.sync.dma_start(g2slabB[rr - RA:rr - RA + 128, 0:CE],
                                            g2row[:])
                      # ---------- Phase D1: AllGather half A of g2'
                      if K_PHASES >= 4 and t == TILES_A - 1:
                          allgather(g2slabA, g2tabA)

                  aggregate_phase(ctx, g1tabA, g1tabB, H_GCN, H_GCN, acc_dC, epi_c, 0)

              # ---------------- Phase D2: AllGather half B of g2'
              if K_PHASES >= 4:
                  allgather(g2slabB, g2tabB)

              # ---------------- Phase E: L2 aggregate -> out
              if K_PHASES >= 5:
                with ExitStack() as ctx:
                  hpool = ctx.enter_context(tc.tile_pool(name="outp", bufs=3))

                  def epi_e(t, pa):
                      ot = hpool.tile([128, CE], f32, tag="ot")
                      nc.scalar.activation(ot[:], pa[:],
                                           mybir.ActivationFunctionType.Copy,
                                           bias=0.0, scale=diso_sb[:, t:t + 1])
                      ot2 = hpool.tile([128, CE], f32, tag="ot2")
                      nc.vector.tensor_add(ot2[:], ot[:], b2_sb[:])
                      nc.sync.dma_start(t_out[t * 128:(t + 1) * 128, :],
                                        ot2[:, :N_CLS])

                  aggregate_phase(ctx, g2tabA, g2tabB, C_PAD, CE, acc_dE, epi_e, 1)

            for _rep in range(reps):
                _body()

    nc.compile()
    _split_multi_waits(nc)
    return nc, N_tg


def _split_multi_waits(nc, max_waits=1):
    """walrus CoreV3 rejects >max_waits sem waits on one instruction; split
    extras onto preceding NOPs on the same engine."""
    n = 0
    for fn in nc.m.functions:
        for bb in fn.blocks:
            insts = bb.instructions
            i = 0
            while i < len(insts):
                inst = insts[i]
                si = inst.sync_info
                if si is not None and si.on_wait and len(si.on_wait) > max_waits:
                    waits = list(si.on_wait)
                    keep = waits[-max_waits:]
                    extra = waits[:-max_waits]
                    new_insts = []
                    for cs in range(0, len(extra), max_waits):
                        nop = mybir.InstNoOp(
                            name=f"I-waitsplit-{id(inst)}-{cs}-{n}",
                            sync_info=mybir.SyncInfo(
                                on_wait=extra[cs:cs + max_waits], on_update=[]),
                            bass_nofuse=True,
                            engine=inst.engine)
                        new_insts.append(nop)
                        n += 1
                    si.on_wait = keep
                    for j, nop in enumerate(new_insts):
                        insts.insert(i + j, nop)
                    i += len(new_insts)
                i += 1
    return n


def prepare(x, edge_index, W_mlp, b_mlp, W1, b1, W2, b2, reps=None, k_phases=None):
    x = np.asarray(x, dtype=np.float32)
    W_mlp_ = np.asarray(W_mlp, dtype=np.float32)
    b_mlp_ = np.asarray(b_mlp, dtype=np.float32)
    W1_ = np.asarray(W1, dtype=np.float32)
    b1_ = np.asarray(b1, dtype=np.float32)
    W2_ = np.asarray(W2, dtype=np.float32)
    b2_ = np.asarray(b2, dtype=np.float32)

    if reps is None:
        reps = int(os.environ.get("KREPS", "1"))
    G, off_ts, chunk_off, total, gidx, dloc, diso, cntc = _preprocess(edge_index)
    nc, _ = _build_program(G, off_ts, chunk_off, total,
                           b1_nonzero=bool(np.abs(b1_).max() > 0), reps=reps,
                           k_phases=k_phases)

    W2p = np.zeros((H_GCN, CE), np.float32)
    W2p[:, :N_CLS] = W2_
    b2p = np.zeros(CE, np.float32)
    b2p[:N_CLS] = b2_
    bmlp_pk = np.ascontiguousarray(b_mlp_.reshape(2, 128).T)
    b1bc = np.tile(b1_[None, :], (128, 1)).astype(np.float32)
    b2bc = np.tile(b2p[None, :], (128, 1)).astype(np.float32)
    chmax = int(((G + 127) // 128).max())
    iota = np.tile(np.arange(128, dtype=np.float32)[None, :],
                   (128, chmax)).astype(MM_NP)
    ident = np.eye(128, dtype=MM_NP)

    in_maps = []
    for c in range(NCORES):
        xs = np.zeros((RPAD, N_FEAT), np.float32)
        xs[:RPC] = x[c * RPC:(c + 1) * RPC]
        in_maps.append({
            "xT": np.ascontiguousarray(xs.T).astype(MM_NP),
            "wmlp": W_mlp_.astype(MM_NP), "w1": W1_.astype(MM_NP),
            "w2": W2p.astype(MM_NP),
            "bmlp": bmlp_pk, "b1bc": b1bc, "b2bc": b2bc,
            "iota": iota, "ident": ident,
            "gidx": gidx[c], "dloc": dloc[c].astype(MM_NP), "diso": diso[c],
            "gcnt": np.broadcast_to(
                np.concatenate([cntc[c].reshape(-1), cntc[c].reshape(-1)])
                .astype(np.int32)[None, :], (128, 2 * NTILE * NSEG)).copy(),
        })

    return nc, in_maps


def kernel(**inputs):
    nc, in_maps = prepare(**inputs)
    res = run_bass_kernel_spmd(nc, in_maps, list(range(NCORES)))
    global last_results
    last_results = res
    out = np.concatenate(
        [res.results[c]["out"][:RPC] for c in range(NCORES)], axis=0)
    return out.astype(np.float32)


last_results = None


if __name__ == "__main__":
    import reference
    from np_ref import np_reference
    inputs = {k: np.asarray(v) for k, v in reference.setup_inputs().items()}
    got = kernel(**inputs)
    exp = np_reference(**inputs)
    denom = np.abs(exp).max()
    err = np.abs(got - exp).max()
    print(f"abs err {err}  rel err {err / denom}  scale {denom}")

